# revision 28
# baseline (speedup 1.0000x reference)
"""GCN (3-layer GCNConv + 3 FC + log_softmax) on 8 Trainium2 NeuronCores.

Strategy (v4)
-------------
Nodes sharded across 8 cores (6272 rows each, padded 50000 -> 50176).
Per conv layer:

  z = y_prev @ W          (per-core rows, bf16, computed in prev layer's epilogue)
  AllGather z             (two halves: AG-a emitted mid-epilogue, AG-b at end)
  agg[dst] = sum coef * z[src]   (per-core over edges whose dst it owns)
  y = relu(agg + b [+ y1])

The sparse aggregation is edge-chunk matmuls: gathered source rows
G [128e x 128f] (dma_gather from the AG'd z-table in HBM) times a one-hot
scatter matrix S [128e x 128d] with S[e, dst_e] = coef_e, accumulated in
PSUM per destination tile.

Key scheduling facts this version is built around (HW-measured):
 - collective_compute blocks the in-order Pool sequencer until the
   collective completes, but already-queued SWDGE gather descriptors keep
   draining.  So: each layer runs in TWO PHASES (LO = srcs in half a,
   needs AG-a only; HI = srcs in half b, needs AG-b), gathers are split
   into ~12-chunk sub-gathers dispatched 18 deep ahead of the consuming
   matmuls across 4 SWDGE queues, and each AG trigger is emitted behind
   ~16 queued sub-gathers.  This fully hides the 6 AllGathers.
 - DVE ops contend with SWDGE descriptor generation for the shared SBUF
   port pair PER INSTRUCTION (tensor_tensor ~1us/op, tensor_scalar
   ~0.3us/op, independent of op size), so: epilogue adds (LO-partial +
   y1 + fc3 bias) are folded into the PSUM accumulation as identity
   matmuls on the tensor engine, and the one-hot S matrices are built
   16 chunks at a time with two wide tensor_tensor ops (iota==dst
   broadcast, then *coef broadcast) instead of one tensor_scalar per
   chunk — 12x fewer DVE instructions.
 - LO partials spill psum->SBUF (bf16) via the scalar engine, which
   never contends.

Structure (chunk counts per tile/side) is cross-core uniform (max over
cores, padded with coef=0 dummy edges) so a single SPMD program serves
all 8 cores; per-core data (indices, coefs) are kernel inputs.
"""
import math
from dataclasses import dataclass, field

import numpy as np
import ml_dtypes

BF16 = ml_dtypes.bfloat16


@dataclass
class GCfg:
    n: int = 50000              # real node count
    n_feat: int = 128
    n_cores: int = 8
    tiles_per_core: int = 49
    group: int = 4              # dst tiles per psum group (<=4, 4*128 fp32 = 1 bank)
    split: int = 32768          # int16 gather address split
    n_classes: int = 40
    n_layers: int = 3

    @property
    def nsh(self):
        return self.tiles_per_core * 128

    @property
    def npad(self):
        return self.nsh * self.n_cores

    @property
    def n_groups(self):
        return math.ceil(self.tiles_per_core / self.group)


@dataclass
class Plan:
    cfg: GCfg
    k: np.ndarray              # [tiles, 2] chunks per (tile, side)
    groups: list = field(default_factory=list)
    idx_cols: int = 0
    n_chunks: int = 0
    # per-core data
    eidx: list = field(default_factory=list)    # [128, idx_cols] int16
    edst: list = field(default_factory=list)    # [128, n_chunks] f32
    ecoef: list = field(default_factory=list)   # [128, n_chunks] f32


def preprocess(edge_index: np.ndarray, cfg: GCfg) -> Plan:
    n, nsh, npad = cfg.n, cfg.nsh, cfg.npad
    T, G = cfg.tiles_per_core, cfg.group
    NC = cfg.n_cores
    NG = cfg.n_groups

    loop = np.arange(n, dtype=np.int64)
    src = np.concatenate([edge_index[0].astype(np.int64), loop])
    dst = np.concatenate([edge_index[1].astype(np.int64), loop])
    deg = np.bincount(dst, minlength=npad).astype(np.float32)
    deg[deg == 0] = 1.0
    norm = 1.0 / np.sqrt(deg)
    coef = (norm[src] * norm[dst]).astype(np.float32)

    core = dst // nsh
    tile = (dst % nsh) // 128
    dloc = dst % 128
    HA = (T + 1) // 2          # tiles in half a
    RA = HA * 128              # rows per core in half a
    RB = nsh - RA
    side = ((src % nsh) >= RA).astype(np.int64)
    grp = tile // G

    # sort edges by (core, side, group, tile) — SIDE MAJOR (lo/hi phases)
    order = np.lexsort((tile, grp, side, core))
    src, dst, coef = src[order], dst[order], coef[order]
    core, tile, dloc, side = core[order], tile[order], dloc[order], side[order]
    grp = tile // G

    # counts per (core, tile, side); uniform chunk counts k
    bid = (core * T + tile) * 2 + side
    cnt = np.bincount(bid, minlength=NC * T * 2).reshape(NC, T, 2)
    k = np.ceil(cnt.max(axis=0) / 128).astype(np.int64)   # [T, 2]
    k = np.maximum(k, 1)       # every (tile, side) gets >=1 chunk

    # build structure: phase s -> groups -> tiles -> chunks
    groups = [{"tiles": [], "off": {}, "rows": {}} for _ in range(NG)]
    for g in range(NG):
        tiles = list(range(g * G, min((g + 1) * G, T)))
        for q, t in enumerate(tiles):
            groups[g]["tiles"].append({"t": t, "q": q, "chunks": {0: [], 1: []}})
    cid = 0
    idx_col = 0
    pos_off = np.zeros((T, 2), dtype=np.int64)
    for s in (0, 1):
        for g in range(NG):
            groups[g]["off"][s] = idx_col
            slot = 0
            for tinfo in groups[g]["tiles"]:
                t = tinfo["t"]
                pos_off[t, s] = (cid) * 128
                for _ in range(int(k[t, s])):
                    tinfo["chunks"][s].append((slot, cid))
                    slot += 1
                    cid += 1
            groups[g]["rows"][s] = slot * 128
            idx_col += slot * 8      # 128 idx/chunk -> 8 int16 cols/chunk
    n_chunks = cid
    idx_cols = idx_col
    total_pos = n_chunks * 128

    plan = Plan(cfg=cfg, k=k, groups=groups, idx_cols=idx_cols, n_chunks=n_chunks)

    # per-core packed arrays; bucket order (side, grp, tile) matches stream
    K = 2 * NG * T
    okey = (side * NG + grp) * T + tile
    starts = np.searchsorted(core * K + okey, np.arange(NC * K))
    sort_key = core * K + okey
    rank = np.arange(len(src)) - starts[sort_key]

    for c in range(NC):
        m = core == c
        pos = pos_off[tile[m], side[m]] + rank[m]
        idx_full = np.zeros(total_pos, dtype=np.int16)
        dst_full = np.zeros(total_pos, dtype=np.float32)
        coef_full = np.zeros(total_pos, dtype=np.float32)
        sm, sdm = src[m], side[m]
        sv = np.where(sdm == 0,
                      (sm // nsh) * RA + (sm % nsh),
                      (sm // nsh) * RB + (sm % nsh) - RA)
        idx_full[pos] = sv.astype(np.int16)
        dst_full[pos] = dloc[m]
        coef_full[pos] = coef[m]
        # idx wrap: position i -> partition i%16, col i//16, replicated x8
        a16 = idx_full.reshape(-1, 16).T            # [16, total/16]
        eidx = np.tile(a16, (8, 1)).astype(np.int16)
        edst = dst_full.reshape(-1, 128).T.astype(np.float32)   # [128, n_chunks]
        ecoef = coef_full.reshape(-1, 128).T.astype(np.float32)
        plan.eidx.append(np.ascontiguousarray(eidx))
        plan.edst.append(np.ascontiguousarray(edst))
        plan.ecoef.append(np.ascontiguousarray(ecoef))
    return plan


def build_kernel(plan: Plan, n_queues: int = 4, mock_ag: bool = False, no_gather: bool = False, s_on_act: bool = False, act_frac: float = 0.0, csize: int = 4, lookahead: int = 50, agb_pos: int = 42, smp_bufs: int = 8, sbatch: int = 16, pagg_bufs: int = 2, sp_bufs: int = 4, pmm_bufs: int = 2, eq_pool: bool = False):
    import concourse.mybir as mybir
    import concourse.tile as tile
    from concourse import bacc
    from concourse.bass import ts

    cfg = plan.cfg
    NC, T, H = cfg.n_cores, cfg.tiles_per_core, cfg.n_feat
    NCL = cfg.n_classes
    NSH = cfg.nsh
    f32, bf16, i16 = mybir.dt.float32, mybir.dt.bfloat16, mybir.dt.int16
    EQ, MUL, ADD, SUB = (mybir.AluOpType.is_equal, mybir.AluOpType.mult,
                         mybir.AluOpType.add, mybir.AluOpType.subtract)
    AF = mybir.ActivationFunctionType

    nc = bacc.Bacc("TRN2", target_bir_lowering=False, debug=False,
                   num_devices=NC, num_swdge_queues=n_queues)

    din = {}
    def dram_in(name, shape, dt):
        din[name] = nc.dram_tensor(name, shape, dt, kind="ExternalInput")
        return din[name]

    eidx_d = dram_in("eidx", [128, plan.idx_cols], i16)
    edst_d = (dram_in("negdst", [128, plan.n_chunks], f32) if s_on_act
              else dram_in("edst", [128, plan.n_chunks], f32))
    ecoef_d = dram_in("ecoef", [128, plan.n_chunks], f32)
    use_act = s_on_act or act_frac > 0.0
    ecoefn_d = dram_in("ecoefn", [128, plan.n_chunks], f32) if use_act else None
    negdst2_d = dram_in("negdst", [128, plan.n_chunks], f32) if (use_act and not s_on_act) else None
    ident_d = dram_in("ident", [128, 128], bf16)
    xT_d = dram_in("xT", [128, NSH], bf16)
    w_d = [dram_in(f"w{i}", [H, H], bf16) for i in range(3)]
    b_d = [dram_in(f"b{i}", [H, 1], f32) for i in range(3)]
    fw1_d = dram_in("fw1", [H, H], bf16)
    fw2_d = dram_in("fw2", [H, H], bf16)
    fw3_d = dram_in("fw3", [H, NCL], bf16)
    fb1_d = dram_in("fb1", [H, 1], f32)
    fb2_d = dram_in("fb2", [H, 1], f32)
    fb3_d = dram_in("fb3", [128, NCL], bf16)
    out_d = nc.dram_tensor("out", [NSH, NCL], f32, kind="ExternalOutput")

    HA = (T + 1) // 2
    RA, RB = HA * 128, NSH - HA * 128
    ag_in_a = [nc.dram_tensor(f"ag_ina{i}", [RA, H], bf16, kind="Internal")
               for i in range(3)]
    ag_in_b = [nc.dram_tensor(f"ag_inb{i}", [RB, H], bf16, kind="Internal")
               for i in range(3)]
    ag_out_a = [nc.dram_tensor(f"ag_outa{i}", [RA * NC, H], bf16, kind="Internal",
                               addr_space="Shared") for i in range(3)]
    ag_out_b = [nc.dram_tensor(f"ag_outb{i}", [RB * NC, H], bf16, kind="Internal",
                               addr_space="Shared") for i in range(3)]

    def z_dst(L, t):
        if t < HA:
            return ag_in_a[L].ap()[t * 128:(t + 1) * 128, :]
        return ag_in_b[L].ap()[(t - HA) * 128:(t - HA + 1) * 128, :]

    def emit_ag(L, half):
        i, o = (ag_in_a, ag_out_a) if half == 0 else (ag_in_b, ag_out_b)
        if mock_ag:
            nc.sync.dma_start(out=o[L].ap()[0:i[L].shape[0], :], in_=i[L].ap())
            return
        nc.gpsimd.collective_compute(
            "AllGather", mybir.AluOpType.bypass,
            replica_groups=[list(range(NC))],
            ins=[i[L].ap()], outs=[o[L].ap()])

    # phase chunk ranges: phase 0 cids [0, p0), phase 1 [p0, n_chunks)
    p0 = int(plan.k[:, 0].sum())
    phase_base = {0: 0, 1: p0}
    phase_n = {0: p0, 1: plan.n_chunks - p0}

    gq = [0]
    def next_q():
        q = gq[0] % n_queues
        gq[0] += 1
        return q

    with tile.TileContext(nc) as tc:
        with (
            tc.tile_pool(name="const", bufs=1) as cp,
            tc.tile_pool(name="gsub", bufs=lookahead + 2, space="SBUF") as gsub,
            tc.tile_pool(name="sb", bufs=sp_bufs) as sp,
            tc.tile_pool(name="smat", bufs=smp_bufs) as smp,
            tc.tile_pool(name="seqp", bufs=3) as seqp,
            tc.tile_pool(name="pagg", bufs=pagg_bufs, space="PSUM") as pagg,
            tc.tile_pool(name="pmm", bufs=pmm_bufs, space="PSUM") as pmm,
        ):
            # ---- resident constants ----
            eidx = cp.tile([128, plan.idx_cols], i16, tag="eidx")
            nc.sync.dma_start(out=eidx[:], in_=eidx_d.ap())
            edst = cp.tile([128, plan.n_chunks], f32, tag="edst")
            nc.sync.dma_start(out=edst[:], in_=edst_d.ap())
            ecoef = cp.tile([128, plan.n_chunks], f32, tag="ecoef")
            nc.sync.dma_start(out=ecoef[:], in_=ecoef_d.ap())
            ident = cp.tile([128, 128], bf16, tag="ident")
            nc.sync.dma_start(out=ident[:], in_=ident_d.ap())
            if use_act:
                ecoefn = cp.tile([128, plan.n_chunks], f32, tag="ecoefn")
                nc.sync.dma_start(out=ecoefn[:], in_=ecoefn_d.ap())
            if use_act and not s_on_act:
                negdst = cp.tile([128, plan.n_chunks], f32, tag="negdst")
                nc.sync.dma_start(out=negdst[:], in_=negdst2_d.ap())
            xT = cp.tile([128, NSH], bf16, tag="xT")
            nc.sync.dma_start(out=xT[:], in_=xT_d.ap())
            ws = []
            for i in range(3):
                w = cp.tile([H, H], bf16, tag=f"w{i}")
                nc.sync.dma_start(out=w[:], in_=w_d[i].ap())
                ws.append(w)
            bs = []
            for i in range(3):
                b = cp.tile([H, 1], f32, tag=f"b{i}")
                nc.sync.dma_start(out=b[:], in_=b_d[i].ap())
                bs.append(b)
            fw1 = cp.tile([H, H], bf16, tag="fw1")
            nc.sync.dma_start(out=fw1[:], in_=fw1_d.ap())
            fw2 = cp.tile([H, H], bf16, tag="fw2")
            nc.sync.dma_start(out=fw2[:], in_=fw2_d.ap())
            fw3 = cp.tile([H, NCL], bf16, tag="fw3")
            nc.sync.dma_start(out=fw3[:], in_=fw3_d.ap())
            fb1 = cp.tile([H, 1], f32, tag="fb1")
            nc.sync.dma_start(out=fb1[:], in_=fb1_d.ap())
            fb2 = cp.tile([H, 1], f32, tag="fb2")
            nc.sync.dma_start(out=fb2[:], in_=fb2_d.ap())
            fb3 = cp.tile([128, NCL], bf16, tag="fb3")
            nc.sync.dma_start(out=fb3[:], in_=fb3_d.ap())

            iota = cp.tile([128, 128], bf16, tag="iota")
            nc.gpsimd.iota(iota[:], pattern=[[1, 128]], base=0,
                           channel_multiplier=0,
                           allow_small_or_imprecise_dtypes=True)
            iotak = cp.tile([128, sbatch, 128], bf16, tag="iotak")
            nc.gpsimd.iota(iotak[:], pattern=[[0, sbatch], [1, 128]], base=0,
                           channel_multiplier=0,
                           allow_small_or_imprecise_dtypes=True)
            y1 = cp.tile([128, NSH], bf16, tag="y1")
            accum = cp.tile([128, NSH], bf16, tag="accum")

            # ---- z0 = x @ W0 (own rows), scatter to ag_in[0] ----
            for t in range(T):
                psz = pmm.tile([128, H], f32, tag="pz")
                nc.tensor.matmul(out=psz[:], lhsT=xT[:, ts(t, 128)],
                                 rhs=ws[0][:], start=True, stop=True)
                zt = sp.tile([128, H], bf16, tag="zt")
                nc.scalar.activation(out=zt[:], in_=psz[:], func=AF.Copy)
                nc.sync.dma_start(out=z_dst(0, t), in_=zt[:])
                if t == HA - 1:
                    emit_ag(0, 0)

            # ---- conv layers: two phases, sub-split pipelined gathers ----
            CSIZE = csize       # chunks per sub-gather (csize*128 rows)
            LOOKAHEAD = lookahead
            AGB_POS = agb_pos   # LO sub-gathers queued before AG-b trigger

            for L in range(3):
                ztabs = {0: ag_out_a[L].ap(), 1: ag_out_b[L].ap()}
                for s in (0, 1):
                    base, nph = phase_base[s], phase_n[s]
                    n_sub = (nph + CSIZE - 1) // CSIZE
                    subs = []
                    state = {"agb": s != 0}

                    def emit_subg():
                        i = len(subs)
                        c0 = i * CSIZE
                        ncs = min(CSIZE, nph - c0)
                        gt = gsub.tile([128, CSIZE, 128], bf16, tag="gsub")
                        cg = base + c0
                        if no_gather:
                            nc.vector.tensor_copy(out=gt[:, 0, :], in_=iota[:])
                        else:
                         nc.gpsimd.dma_gather(
                            out_ap=gt[:, :ncs, :], in_ap=ztabs[s][:, :],
                            idxs_ap=eidx[:, cg * 8:(cg + ncs) * 8],
                            num_idxs=ncs * 128, num_idxs_reg=ncs * 128,
                            elem_size=H, single_packet=False,
                            queue_num=next_q())
                        subs.append(gt)
                        if not state["agb"] and len(subs) > min(AGB_POS, n_sub - 1):
                            emit_ag(L, 1)
                            state["agb"] = True

                    def ensure_subg(rel_cid):
                        need = min(rel_cid // CSIZE + LOOKAHEAD, n_sub - 1)
                        while len(subs) <= need:
                            emit_subg()

                    def lhs_for(cid):
                        rel = cid - base
                        return subs[rel // CSIZE][:, rel % CSIZE, :]

                    sbatches = []
                    n_sb = (nph + sbatch - 1) // sbatch

                    def emit_sbatch():
                        i = len(sbatches)
                        b0 = i * sbatch
                        kb = min(sbatch, nph - b0)
                        cg = base + b0
                        eqp = seqp if eq_pool else smp
                        eq = eqp.tile([128, sbatch, 128], bf16, tag="Seq")
                        nc.vector.tensor_tensor(
                            out=eq[:, :kb, :], in0=iotak[:, :kb, :],
                            in1=edst[:, cg:cg + kb].unsqueeze(-1)
                                .broadcast_to((128, kb, 128)),
                            op=EQ)
                        Sb = smp.tile([128, sbatch, 128], bf16, tag="Sb")
                        nc.vector.tensor_tensor(
                            out=Sb[:, :kb, :], in0=eq[:, :kb, :],
                            in1=ecoef[:, cg:cg + kb].unsqueeze(-1)
                                .broadcast_to((128, kb, 128)),
                            op=MUL)
                        sbatches.append(Sb)

                    def S_for(cid):
                        rel = cid - base
                        while len(sbatches) <= rel // sbatch:
                            emit_sbatch()
                        return sbatches[rel // sbatch][:, rel % sbatch, :]

                    for g in plan.groups:
                        ps = pagg.tile([128, 512], f32, tag="pagg")
                        for tinfo in g["tiles"]:
                            t, q = tinfo["t"], tinfo["q"]
                            chunks = tinfo["chunks"][s]
                            nch = len(chunks)
                            extra = 0 if s == 0 else (1 + (1 if L > 0 else 0))
                            for j, (slot, cid) in enumerate(chunks):
                                ensure_subg(cid - base)
                                nc.tensor.matmul(
                                    out=ps[:, ts(q, 128)],
                                    lhsT=lhs_for(cid), rhs=S_for(cid),
                                    start=(j == 0),
                                    stop=(j == nch - 1 and extra == 0),
                                    skip_group_check=True)
                            if s == 0:
                                continue
                            # ---- epilogue for tile t (HI phase) ----
                            # accum (+ y1) folded into the psum group on PE
                            nc.tensor.matmul(
                                out=ps[:, ts(q, 128)], lhsT=ident[:],
                                rhs=accum[:, ts(t, 128)],
                                start=False, stop=(L == 0),
                                skip_group_check=True)
                            if L > 0:
                                nc.tensor.matmul(
                                    out=ps[:, ts(q, 128)], lhsT=ident[:],
                                    rhs=y1[:, ts(t, 128)],
                                    start=False, stop=True,
                                    skip_group_check=True)
                            if L == 0:
                                nc.scalar.activation(
                                    out=y1[:, ts(t, 128)], in_=ps[:, ts(q, 128)],
                                    func=AF.Relu, bias=bs[0][:])
                                ysrc = y1[:, ts(t, 128)]
                            else:
                                yt = sp.tile([128, 128], bf16, tag="yt")
                                nc.scalar.activation(out=yt[:],
                                                     in_=ps[:, ts(q, 128)],
                                                     func=AF.Relu, bias=bs[L][:])
                                ysrc = yt[:]
                            if L < 2:
                                psz = pmm.tile([128, H], f32, tag="pz")
                                nc.tensor.matmul(out=psz[:], lhsT=ysrc,
                                                 rhs=ws[L + 1][:],
                                                 start=True, stop=True)
                                zt = sp.tile([128, H], bf16, tag="zt")
                                nc.scalar.activation(out=zt[:], in_=psz[:],
                                                     func=AF.Copy)
                                nc.sync.dma_start(out=z_dst(L + 1, t), in_=zt[:])
                                if t == HA - 1:
                                    emit_ag(L + 1, 0)
                            else:
                                # ---- FC head, per tile ----
                                ph = pmm.tile([128, H], f32, tag="pz")
                                nc.tensor.matmul(out=ph[:], lhsT=fw1[:], rhs=ysrc,
                                                 start=True, stop=True)
                                h1 = sp.tile([128, H], bf16, tag="h1")
                                nc.scalar.activation(out=h1[:], in_=ph[:],
                                                     func=AF.Relu, bias=fb1[:])
                                ph2 = pmm.tile([128, H], f32, tag="pz")
                                nc.tensor.matmul(out=ph2[:], lhsT=fw2[:], rhs=h1[:],
                                                 start=True, stop=True)
                                h2 = sp.tile([128, H], bf16, tag="h2")
                                nc.scalar.activation(out=h2[:], in_=ph2[:],
                                                     func=AF.Relu, bias=fb2[:])
                                p3 = pmm.tile([128, NCL], f32, tag="pz")
                                nc.tensor.matmul(out=p3[:], lhsT=h2[:], rhs=fw3[:],
                                                 start=True, stop=False,
                                                 skip_group_check=True)
                                nc.tensor.matmul(out=p3[:], lhsT=ident[:],
                                                 rhs=fb3[:, 0:NCL],
                                                 start=False, stop=True,
                                                 skip_group_check=True)
                                mneg = sp.tile([128, 1], f32, tag="mneg")
                                nc.vector.tensor_reduce(
                                    out=mneg[:], in_=p3[:],
                                    axis=mybir.AxisListType.X,
                                    op=mybir.AluOpType.max, negate=True)
                                ex = sp.tile([128, NCL], f32, tag="ex")
                                ssum = sp.tile([128, 1], f32, tag="ssum")
                                nc.scalar.activation(out=ex[:], in_=p3[:],
                                                     func=AF.Exp, bias=mneg[:],
                                                     accum_out=ssum[:])
                                lg = sp.tile([128, 1], f32, tag="lg")
                                nc.scalar.activation(out=lg[:], in_=ssum[:],
                                                     func=AF.Ln)
                                ot = sp.tile([128, NCL], f32, tag="ot")
                                nc.vector.tensor_scalar(
                                    out=ot[:], in0=p3[:], scalar1=mneg[:],
                                    scalar2=lg[:], op0=ADD, op1=SUB)
                                nc.sync.dma_start(out=out_d.ap()[ts(t, 128), :],
                                                  in_=ot[:])
                        if s == 0:
                            t0c = g["tiles"][0]["t"] * 128
                            ncols = len(g["tiles"]) * 128
                            nc.scalar.activation(out=accum[:, t0c:t0c + ncols],
                                                 in_=ps[:, 0:ncols], func=AF.Copy)
                    if s == 0 and not state["agb"]:
                        emit_ag(L, 1)
                        state["agb"] = True

    nc.compile()
    return nc


def make_in_maps(inputs, plan: Plan):
    cfg = plan.cfg
    NC, NSH, H = cfg.n_cores, cfg.nsh, cfg.n_feat
    x = np.asarray(inputs["x"], dtype=np.float32)
    xp = np.zeros((cfg.npad, H), dtype=np.float32)
    xp[:x.shape[0]] = x
    Wc = np.asarray(inputs["Wconv"], dtype=np.float32)
    bc = np.asarray(inputs["bconv"], dtype=np.float32)
    in_maps = []
    for c in range(NC):
        m = {
            "eidx": plan.eidx[c],
            "edst": plan.edst[c],
            "ecoef": plan.ecoef[c],
            "xT": np.ascontiguousarray(xp[c * NSH:(c + 1) * NSH].T).astype(BF16),
            "fw1": np.asarray(inputs["fc1_w"], np.float32).astype(BF16),
            "fw2": np.asarray(inputs["fc2_w"], np.float32).astype(BF16),
            "fw3": np.asarray(inputs["fc3_w"], np.float32).astype(BF16),
            "fb1": np.asarray(inputs["fc1_b"], np.float32).reshape(H, 1),
            "fb2": np.asarray(inputs["fc2_b"], np.float32).reshape(H, 1),
            "fb3": np.tile(np.asarray(inputs["fc3_b"], np.float32)[None, :],
                           (128, 1)).astype(BF16),
            "ident": np.eye(128, dtype=np.float32).astype(BF16),
            "negdst": -plan.edst[c],
            "ecoefn": -plan.ecoef[c],
        }
        for i in range(3):
            m[f"w{i}"] = Wc[i].astype(BF16)
            m[f"b{i}"] = bc[i].reshape(H, 1).astype(np.float32)
        in_maps.append(m)
    return in_maps


_CACHE = {}


def kernel(**inputs) -> np.ndarray:
    cfg = GCfg()
    edge_index = np.asarray(inputs["edge_index"])
    # cache plan + compiled kernel across calls (same graph => same plan)
    fp = (edge_index.shape, int(edge_index[:, ::10007].sum()),
          int(edge_index[:, 1::9973].sum()))
    if _CACHE.get("fp") != fp:
        plan = preprocess(edge_index, cfg)
        _CACHE.clear()
        _CACHE["fp"] = fp
        _CACHE["plan"] = plan
        _CACHE["nc"] = build_kernel(plan)
    plan = _CACHE["plan"]
    nc = _CACHE["nc"]
    in_maps = make_in_maps(inputs, plan)
    from concourse.bass_utils import run_bass_kernel_spmd
    res = run_bass_kernel_spmd(nc, in_maps, core_ids=list(range(cfg.n_cores)))
    out = np.concatenate([res.results[c]["out"] for c in range(cfg.n_cores)], axis=0)
    return np.ascontiguousarray(out[:cfg.n, :cfg.n_classes].astype(np.float32))
